# revision 43
# baseline (speedup 1.0000x reference)
"""AtrousFourWayMamba Trainium2 kernel (8-core SPMD, d_inner-sharded). v2.

Self-contained: hardcodes all shapes. Accepts FULL inputs, returns FULL output.

Per core c (d-slice of 128 channels):
- warmup AllReduce issued first to absorb collective-engine init / launch skew
- in_proj column-parallel GEMM on PE -> x_c, z_c (128, 2000)
- scan order C (original) needs no reorder: x_c allocated with a 3-col zero
  prefix so the causal conv reads it directly; A/B orders via Pool strided
  copies. Causal conv taps on DVE, SiLU on ACT, x_proj partials on PE,
  one AllReduce per scan (C first so its scan starts earliest).
- selective scan in channel-major layout (partition = channel, tile = state):
    ACT computes da=exp(A[:,n]*delta) straight from delta (per-partition
    scale; no replication); Pool computes dbu = du * B_rep (B row n
    replicated across partitions by DMA from the AR output in DRAM);
    DVE runs tensor_tensor_scan over L=2000; DVE multiplies h*C_rep (bf16);
    PE accumulates y += I @ g_n into PSUM per 500-col chunk.
- BiAttn: LN stats via packed PE transposes + tiny AllReduce; gate
  projections on PE with another tiny AllReduce; att_out and out_proj fused
  into W_comb (bf16 GEMM) scaled by attn so ori chunks feed the final GEMM
  directly; ReduceScatter finishes the kernel.
- host: concatenate the 8 RS shards + add (ob @ out_proj_w.T).
"""
import os
import sys
import types
import ctypes
import contextlib
from contextlib import ExitStack

sys.path.insert(0, '/opt/trn_rl_repo')

import numpy as np


def _install_axon_hooks_shim():
    """antenv.axon_hooks is missing in this image; recreate it so
    run_bass_kernel_spmd(trace=True) can drive NTFF profiling."""
    try:
        from antenv.axon_hooks import get_axon_ntff_profile_hook  # noqa
        return
    except ImportError:
        pass
    so_path = "/opt/axon/libaxon_pjrt.so"
    hook = None
    if os.path.exists(so_path):
        lib = ctypes.CDLL(so_path)
        if hasattr(lib, "axon_start_nrt_profile"):
            lib.axon_start_nrt_profile.argtypes = [ctypes.POINTER(ctypes.c_int64), ctypes.c_size_t]
            lib.axon_start_nrt_profile.restype = ctypes.c_int64
            lib.axon_stop_nrt_profile.argtypes = [ctypes.c_char_p]
            lib.axon_stop_nrt_profile.restype = ctypes.c_int64

            @contextlib.contextmanager
            def _hook(output_dir, device_ids):
                import jax
                jax.devices()
                if device_ids:
                    ids = (ctypes.c_int64 * len(device_ids))(*device_ids)
                    rc = lib.axon_start_nrt_profile(ids, len(device_ids))
                else:
                    rc = lib.axon_start_nrt_profile(None, 0)
                if rc != 0:
                    raise RuntimeError(f"axon_start_nrt_profile rc={rc}")
                try:
                    yield
                finally:
                    n = lib.axon_stop_nrt_profile(str(output_dir).encode())
                    print(f"profile: {n} file(s) written to {output_dir}", file=sys.stderr)

            hook = _hook
    import antenv
    mod = types.ModuleType("antenv.axon_hooks")
    mod.get_axon_ntff_profile_hook = lambda: hook
    mod.set_axon_ntff_profile_hook = lambda h: None
    sys.modules["antenv.axon_hooks"] = mod
    antenv.axon_hooks = mod


_install_axon_hooks_shim()

import concourse.bass as bass
import concourse.bacc as bacc
import concourse.tile as tile
from concourse import mybir
from concourse.bass_utils import run_bass_kernel_spmd

F32 = mybir.dt.float32
BF16 = mybir.dt.bfloat16
FP16 = mybir.dt.float16
AF = mybir.ActivationFunctionType
OP = mybir.AluOpType
AX = mybir.AxisListType

NC_ = 8            # cores
DM = 512           # d_model
DI = 1024          # d_inner
DL = DI // NC_     # 128 channels per core
NS = 16            # d_state
RK = 32            # dt_rank
L = 2000
CH = 500           # l-chunk (one psum bank)
NQ = L // CH

SCAN_ORDER = [2, 1, 0]   # process C (no reorder) first, then B, A

_CACHE = {}


def _seq_views(x, scan, pre=3):
    """Strided-AP pieces of x (128, pre+2000) giving the scan-order sequence.

    x has a `pre`-col zero prefix; views index the data part.
    Returns list of (col_slice, view) covering [0, 2000); each view is a
    (128, a, b) AP whose row-major traversal is that sequence segment.
    """
    v = x[:, pre:].rearrange("p (h w) -> p h w", w=10)
    if scan == 0:   # A: [x0 | x1^T | rev x2 | rev x3^T]
        return [
            (slice(0, 500), v[:, 0::2, 0::2]),
            (slice(500, 1000), v[:, 1::2, 0::2].rearrange("p h w -> p w h")),
            (slice(1000, 1500), v[:, 198::-2, 9::-2]),
            (slice(1500, 2000), v[:, 199::-2, 9::-2].rearrange("p h w -> p w h")),
        ]
    elif scan == 1:  # B: [even rows | odd rows]
        return [
            (slice(0, 1000), v[:, 0::2, :]),
            (slice(1000, 2000), v[:, 1::2, :]),
        ]
    raise ValueError(scan)


def _r3(ap2d, a, b):
    """View a contiguous (128, a*b) AP as (128, a, b)."""
    return ap2d.rearrange("p (a b) -> p a b", b=b)


def _build(dbg=False):
    nc = bacc.Bacc("TRN2", target_bir_lowering=False, debug=False, num_devices=NC_)

    def dump(name, ap):
        if not dbg:
            return
        d = nc.dram_tensor(f"dbg_{name}", list(ap.shape), ap.dtype,
                           kind="ExternalOutput").ap()
        nc.sync.dma_start(d, ap)

    def din(name, shape, dt=F32):
        return nc.dram_tensor(name, list(shape), dt, kind="ExternalInput").ap()

    io = dict(
        hidT=din("hidT", (DM, L), BF16),
        wxT=din("wxT", (DM, DL), BF16),
        wzT=din("wzT", (DM, DL), BF16),
        ones_colf=din("ones_colf", (128, 1)),
        identT=din("identT", (128, 128), BF16),
        ln_g_s=din("ln_g_s", (DL, 1)),
        ln_b=din("ln_b", (DL, 1)),
        grw=din("grw", (DL, 512)),
        grbT=din("grbT", (128, 4)),
        cswT=din("cswT", (512, DI), BF16),
        wcombF=din("wcombF", (DI, DM), BF16),
        csbF=din("csbF", (128, 8)),
    )
    for s in range(3):
        io[f"convw{s}"] = din(f"convw{s}", (4 * DL, DL), BF16)
        io[f"convb{s}"] = din(f"convb{s}", (DL, 1))
        io[f"xwT{s}"] = din(f"xwT{s}", (DL, 64), BF16)
        io[f"dtwT{s}"] = din(f"dtwT{s}", (RK, DL), BF16)
        io[f"dtb{s}"] = din(f"dtb{s}", (DL, 1))
        io[f"avec{s}"] = din(f"avec{s}", (DL, NS))
        io[f"ddiag{s}"] = din(f"ddiag{s}", (DL, DL), BF16)
    out_shard = nc.dram_tensor("out_shard", [L // NC_, DM], BF16, kind="ExternalOutput").ap()

    with tile.TileContext(nc) as tc, ExitStack() as ctx:
        cons = ctx.enter_context(tc.tile_pool(name="cons", bufs=1))
        big = ctx.enter_context(tc.tile_pool(name="big", bufs=1))
        work = ctx.enter_context(tc.tile_pool(name="work", bufs=2))
        psum = ctx.enter_context(tc.tile_pool(name="psum", bufs=2, space="PSUM"))
        dram = ctx.enter_context(tc.tile_pool(name="dram", bufs=1, space="DRAM"))

        def load(name, pool=cons, eng=None):
            src = io[name]
            t = pool.tile(list(src.shape), src.dtype, name=f"sb_{name}")
            (eng or nc.sync).dma_start(t[:], src)
            return t

        # ---------- warmup collective: absorb CC init / launch skew ----------
        ar0_in = dram.tile([1, 8], F32)
        ar0_out = dram.tile([1, 8], F32, addr_space="Shared")
        warm = cons.tile([1, 8], F32, name="warm")
        nc.vector.memset(warm[:], 1.0)
        nc.sync.dma_start(ar0_in[:], warm[:])
        nc.gpsimd.collective_compute(
            "AllReduce", OP.add, replica_groups=[list(range(NC_))],
            ins=[ar0_in.opt()], outs=[ar0_out.opt()])

        # ---------- bulk const loads on the Pool software DGE ----------
        # (issued immediately after the warmup trigger; Pool is idle in the
        # head and this keeps the sync/scalar rings free for hidT streaming)
        convw = []
        for s in range(3):
            cw4 = [cons.tile([DL, DL], BF16, name=f"convw{s}_{k}") for k in range(4)]
            for k in range(4):
                nc.sync.dma_start(cw4[k][:], io[f"convw{s}"][128 * k:128 * (k + 1), :])
            convw.append(cw4)
        convb = [load(f"convb{s}") for s in range(3)]
        xwT = [load(f"xwT{s}") for s in range(3)]
        ones_colf = load("ones_colf")
        identT = load("identT")
        dtwT = [load(f"dtwT{s}") for s in range(3)]
        dtb = [load(f"dtb{s}") for s in range(3)]
        avec = [load(f"avec{s}") for s in range(3)]
        ddiag = [load(f"ddiag{s}") for s in range(3)]
        ln_g_s = load("ln_g_s")
        ln_b_sb = load("ln_b")
        grw_sb = load("grw")
        grbT_sb = load("grbT")
        csbF_sb = load("csbF")
        cswT_sb = [cons.tile([128, DI], BF16, name=f"cswT_sb{k}") for k in range(4)]
        for k in range(4):
            nc.sync.dma_start(cswT_sb[k][:], io["cswT"][128 * k:128 * (k + 1), :])
        wcombF = [cons.tile([128, DM], BF16, name=f"wcombF{j}") for j in range(8)]
        for j in range(8):
            nc.sync.dma_start(wcombF[j][:], io["wcombF"][128 * j:128 * (j + 1), :])

        # ---------- collective buffers ----------
        ar1_ins = [dram.tile([64, L], BF16, name=f"ar1_in{s}") for s in range(3)]
        ar1_outs = [dram.tile([64, L], BF16, addr_space="Shared", name=f"ar1_out{s}")
                    for s in range(3)]
        ar2_in = dram.tile([128, 32], F32)
        ar2_out = dram.tile([128, 32], F32, addr_space="Shared")
        ar3_in = dram.tile([1, 512], F32)
        ar3_out = dram.tile([1, 512], F32, addr_space="Shared")
        a2a_in = dram.tile([L, DL], BF16)
        a2a_out = dram.tile([L, DL], BF16)

        # ================= stage 1: xz GEMM =================
        # x_c/z_c have a 3-col zero prefix so the causal conv for scan C (and
        # the xp buffers for A/B) read [k : k+L] directly.
        x_c = big.tile([DL, L + 3], BF16)
        z_c = big.tile([DL, L], BF16)
        nc.vector.memset(x_c[:, 0:3], 0.0)
        wxT_sb = [cons.tile([128, DL], BF16, name=f"wxT_sb{k}") for k in range(4)]
        wzT_sb = [cons.tile([128, DL], BF16, name=f"wzT_sb{k}") for k in range(4)]
        qeng = [nc.sync, nc.scalar, nc.sync, nc.scalar]
        for k in range(4):
            qeng[k].dma_start(wxT_sb[k][:], io["wxT"][128 * k:128 * (k + 1), :])
            qeng[k].dma_start(wzT_sb[k][:], io["wzT"][128 * k:128 * (k + 1), :])
        for q in range(NQ):
            lsl = slice(q * CH, (q + 1) * CH)
            mmx = psum.tile([128, CH], F32, tag="mm", name=f"mmx{q}")
            mmz = psum.tile([128, CH], F32, tag="mm", name=f"mmz{q}")
            for k in range(4):
                hidt = work.tile([128, CH], BF16, tag="hidt", bufs=6, name=f"hidt{q}_{k}")
                if k == 3:
                    # third DMA channel: Pool's software DGE carries one of
                    # the four k-blocks so the sync/scalar rings keep up
                    nc.gpsimd.dma_start(hidt[:], io["hidT"][128 * k:128 * (k + 1), lsl])
                else:
                    nc.sync.dma_start(hidt[0:64, :], io["hidT"][128 * k:128 * k + 64, lsl])
                    nc.scalar.dma_start(hidt[64:128, :], io["hidT"][128 * k + 64:128 * (k + 1), lsl])
                nc.tensor.matmul(mmx[:], wxT_sb[k][:], hidt[:], start=(k == 0), stop=(k == 3))
                nc.tensor.matmul(mmz[:], wzT_sb[k][:], hidt[:], start=(k == 0), stop=(k == 3))
            nc.scalar.copy(x_c[:, 3 + q * CH:3 + (q + 1) * CH], mmx[:])
            nc.vector.tensor_copy(z_c[:, lsl], mmz[:])
        dump("x_c", x_c[:, 3:])
        dump("z_c", z_c[:])

        # ================= stage 2: per-scan conv/silu/x_proj =================
        us, szs = [], []
        xps = {}
        for s in SCAN_ORDER:
            if s < 2:
                xp = big.tile([DL, L + 3], BF16, tag="xp", bufs=2, name=f"xp{s}")
                nc.vector.memset(xp[:, 0:3], 0.0)
                for dsl, view in _seq_views(x_c, s):
                    a, b = view.shape[1], view.shape[2]
                    nc.vector.tensor_copy(_r3(xp[:, 3 + dsl.start:3 + dsl.stop], a, b), view)
            else:
                xp = x_c
            xps[s] = xp
            # conv on PE: 4 accumulating diagonal matmuls per chunk, then
            # SiLU+bias from PSUM casts to bf16 u on ACT
            u = big.tile([DL, L], BF16, name=f"u{s}")
            for q in range(NQ):
                lsl = slice(q * CH, (q + 1) * CH)
                mmc = psum.tile([128, CH], F32, tag="mmc", name=f"mmc{s}_{q}")
                for k in range(4):
                    nc.tensor.matmul(mmc[:], convw[s][k][:],
                                     xp[:, q * CH + k:q * CH + k + CH],
                                     start=(k == 0), stop=(k == 3))
                nc.scalar.activation(u[:, lsl], mmc[:], AF.Silu, bias=convb[s][:])
            dump(f"u{s}", u[:])
            us.append((s, u))
            # x_proj partials -> ar1_in
            for q in range(NQ):
                lsl = slice(q * CH, (q + 1) * CH)
                mm = psum.tile([64, CH], F32, tag="mm", name=f"mmxp{s}_{q}")
                nc.tensor.matmul(mm[:], xwT[s][:], u[:, lsl], start=True, stop=True)
                st = work.tile([64, CH], BF16, tag="xdst", bufs=2, name=f"xdst{s}_{q}")
                nc.vector.tensor_copy(st[:], mm[:])
                (nc.scalar if q % 2 == 0 else nc.sync).dma_start(ar1_ins[s][:, lsl], st[:])
            # per-scan AllReduce: fires as soon as this scan's partials land
            nc.gpsimd.collective_compute(
                "AllReduce", OP.add, replica_groups=[list(range(NC_))],
                ins=[ar1_ins[s].opt()], outs=[ar1_outs[s].opt()])
        us = [u for _, u in sorted(us)]

        # silu(z) in scan order (ACT; needed only at finalize)
        for s in SCAN_ORDER:
            sz = big.tile([DL, L], BF16, name=f"sz{s}")
            if s < 2:
                for dsl, view in _seq_views(z_c, s, pre=0):
                    a, b = view.shape[1], view.shape[2]
                    nc.scalar.activation(_r3(sz[:, dsl], a, b), view, AF.Silu)
            else:
                nc.scalar.activation(sz[:], z_c[:], AF.Silu)
            dump(f"sz{s}", sz[:])
            szs.append((s, sz))
        szs = [z for _, z in sorted(szs)]

        # preload the Exp ACT table during the head's idle ACT window
        exp_warm = work.tile([1, 1], F32, tag="expw", bufs=1)
        nc.scalar.activation(exp_warm[:], warm[0:1, 0:1], AF.Exp)

        # ================= stage 3: scans (channel-major layout) =============
        def phase_a(s):
            """dt GEMM + softplus -> delta; du = delta*u (Pool); C cast->bf16."""
            bc = big.tile([RK, L], BF16, tag="bc", bufs=1, name=f"bc{s}")
            # dt rows streamed per chunk so the dt GEMM starts immediately
            for q in range(NQ):
                lsl = slice(q * CH, (q + 1) * CH)
                nc.scalar.dma_start(bc[:, lsl], ar1_outs[s][0:RK, lsl])
            # softplus(x) = ln(1 + exp(x)) (no native Softplus table)
            delta = big.tile([DL, L], BF16, tag="delta", bufs=2, name=f"delta{s}")
            for q in range(NQ):
                lsl = slice(q * CH, (q + 1) * CH)
                mm = psum.tile([128, CH], F32, tag="mm", name=f"mmdt{s}_{q}")
                nc.tensor.matmul(mm[:], dtwT[s][:], bc[0:RK, lsl], start=True, stop=True)
                nc.scalar.activation(delta[:, lsl], mm[:], AF.Exp, bias=dtb[s][:])
                nc.scalar.activation(delta[:, lsl], delta[:, lsl], AF.Ln, bias=1.0)
            du = big.tile([DL, L], BF16, tag="du", bufs=2, name=f"du{s}")
            nc.vector.tensor_tensor(du[:], delta[:], us[s][:], OP.mult)
            dump(f"delta{s}", delta[:])
            return delta, du

        ys = [None, None, None]
        scratch_ref = [None]
        deltas = {}
        for idx, s in enumerate(SCAN_ORDER):
            if idx == 0:
                deltas[s] = phase_a(s)
            delta, du = deltas[s]
            ypsum = psum.tile([128, 2048], F32, tag="y", bufs=1, name=f"ypsum{s}")
            for n in range(NS):
                # B / C rows replicated across 128 partitions (bf16 bounce)
                bb = work.tile([128, L], BF16, tag="bb", bufs=3, name=f"bb{s}_{n}")
                b_src = bass.AP(ar1_outs[s].tensor, (RK + n) * L, [[0, 128], [1, L]])
                nc.sync.dma_start(bb[:], b_src)
                cc = work.tile([128, L], BF16, tag="cc", bufs=3, name=f"cc{s}_{n}")
                c_src = bass.AP(ar1_outs[s].tensor, (RK + NS + n) * L, [[0, 128], [1, L]])
                nc.scalar.dma_start(cc[:], c_src)
                da = work.tile([128, L], FP16, tag="da", bufs=3, name=f"da{s}_{n}")
                nc.scalar.activation(da[:], delta[:], AF.Exp, scale=avec[s][:, n:n + 1])
                dbu = work.tile([128, L], BF16, tag="dbu", bufs=3, name=f"dbu{s}_{n}")
                nc.vector.tensor_tensor(dbu[:], bb[:], du[:], OP.mult)
                h = work.tile([128, L], BF16, tag="h", bufs=3, name=f"h{s}_{n}")
                nc.vector.tensor_tensor_scan(h[:], da[:], dbu[:], 0.0, OP.mult, OP.add)
                if s == SCAN_ORDER[0] and n == 0:
                    dump("h00", h[:])
                g = work.tile([128, L], BF16, tag="g", bufs=3, name=f"g{s}_{n}")
                nc.vector.tensor_tensor(g[:], h[:], cc[:], OP.mult)
                for q in range(NQ):
                    lsl = slice(q * CH, (q + 1) * CH)
                    # 512-aligned output slice: one matmul fits one psum bank
                    nc.tensor.matmul(ypsum[:, 512 * q:512 * q + CH],
                                     identT[:], g[:, lsl],
                                     start=(n == 0), stop=False)
                if n == NS - 1:
                    # fold D*u into the same accumulation (diagonal weights)
                    for q in range(NQ):
                        lsl = slice(q * CH, (q + 1) * CH)
                        nc.tensor.matmul(ypsum[:, 512 * q:512 * q + CH],
                                         ddiag[s][:], us[s][:, lsl],
                                         start=False, stop=True)
                if n == 3 and idx + 1 < 3:
                    # issue next scan's phase A mid-flight so its PE/ACT/Pool
                    # work fills gaps and delta is ready when tiles start
                    deltas[SCAN_ORDER[idx + 1]] = phase_a(SCAN_ORDER[idx + 1])
            y = big.tile([DL, L], BF16, name=f"y{s}")
            yp_view = ypsum.rearrange("p (a b) -> p a b", b=512)[:, :, 0:CH]
            nc.vector.tensor_tensor(_r3(y[:], NQ, CH), yp_view,
                                    _r3(szs[s][:], NQ, CH), OP.mult)
            dump(f"ys{s}", y[:])
            ys[s] = y
            if ys[1] is not None and ys[2] is not None and scratch_ref[0] is None:
                scratch = big.tile([DL, L], BF16, tag="scr8k", name="scratch")
                scratch_ref[0] = scratch
                vT = scratch.rearrange("p (h w) -> p h w", w=10)
                vyC = ys[2].rearrange("p (h w) -> p h w", w=10)
                for p_ in range(2):
                    nc.vector.tensor_tensor(
                        vT[:, p_::2, :], _r3(ys[1][:, 1000 * p_:1000 * (p_ + 1)], 100, 10),
                        vyC[:, p_::2, :], OP.add)

        # preload the Sqrt ACT table while the last scan drains
        sq_warm = work.tile([1, 1], F32, tag="sqw", bufs=1)
        nc.scalar.activation(sq_warm[:], warm[0:1, 0:1], AF.Sqrt)

        # ================= stage 4: un-permute & sum (vT done mid-phase) ====
        ori = big.tile([DL, L], BF16)
        scratch = scratch_ref[0]
        vC = ori.rearrange("p (h w) -> p h w", w=10)
        vT = scratch.rearrange("p (h w) -> p h w", w=10)
        yA = ys[0]
        q00 = _r3(yA[:, 0:500], 100, 5)
        q10 = yA[:, 500:1000].rearrange("p (w h) -> p h w", h=100)
        q01 = yA[:, 1000:1500][:, ::-1].rearrange("p (h w) -> p h w", w=5)
        q11 = yA[:, 1500:2000][:, ::-1].rearrange("p (w h) -> p h w", h=100)
        nc.vector.tensor_tensor(vC[:, 0::2, 0::2], vT[:, 0::2, 0::2], q00, OP.add)
        nc.vector.tensor_tensor(vC[:, 1::2, 0::2], vT[:, 1::2, 0::2], q10, OP.add)
        nc.vector.tensor_tensor(vC[:, 0::2, 1::2], vT[:, 0::2, 1::2], q01, OP.add)
        nc.vector.tensor_tensor(vC[:, 1::2, 1::2], vT[:, 1::2, 1::2], q11, OP.add)

        dump("ori", ori[:])
        # ================= stage 5: BiAttn =================
        # Stats in transposed (l-on-partitions) layout: 16 PE transposes of
        # (128, 125) ori chunks packed 4-per-psum-bank; per-l scalars live in
        # (128, 16) tiles.
        LC = 125  # transposed-chunk width (16 x 125 = 2000 l's)
        oriT = big.tile([128, 16 * 128], BF16)    # all 16 transposed chunks
        s1T = cons.tile([128, 16], F32)
        s2T = cons.tile([128, 16], F32)
        # only partitions 0..124 get written by the reduces; zero the rest so
        # the murX partition-reduction doesn't pick up garbage
        nc.vector.memset(s1T[:], 0.0)
        nc.vector.memset(s2T[:], 0.0)
        for gidx in range(4):
            otp = psum.tile([LC, 512], BF16, tag="mm", name=f"otp{gidx}")
            for j in range(4):
                lc = gidx * 4 + j
                csl = slice(LC * lc, LC * (lc + 1))
                nc.tensor.transpose(otp[:, 128 * j:128 * (j + 1)], ori[:, csl], identT[:])
            osl = slice(512 * gidx, 512 * (gidx + 1))
            nc.scalar.copy(oriT[0:LC, osl], otp[:])
            # stream this transposed chunk out for the AllToAll: element
            # [l, 128*j + d] of the SBUF slice lands at a2a_in[500g+125j+l, d]
            a2a_dst = bass.AP(a2a_in.tensor, 500 * gidx * DL,
                              [[DL, LC], [LC * DL, 4], [1, DL]])
            (nc.sync if gidx % 2 == 0 else nc.scalar).dma_start(
                a2a_dst, _r3(oriT[0:LC, osl], 4, DL))
            s1o = s1T[0:LC, 4 * gidx:4 * gidx + 4].rearrange("p (a b) -> p a b", b=1)
            nc.vector.tensor_reduce(s1o, _r3(otp[:], 4, 128), axis=AX.X, op=OP.add)
            sq = work.tile([LC, 512], BF16, tag="sq", bufs=2, name=f"sq{gidx}")
            nc.vector.tensor_tensor(sq[:], oriT[0:LC, osl], oriT[0:LC, osl], OP.mult)
            s2o = s2T[0:LC, 4 * gidx:4 * gidx + 4].rearrange("p (a b) -> p a b", b=1)
            nc.vector.tensor_reduce(s2o, _r3(sq[:], 4, 128), axis=AX.X, op=OP.add)
        nc.sync.dma_start(ar2_in[:, 0:16], s1T[:])
        nc.sync.dma_start(ar2_in[:, 16:32], s2T[:])
        nc.gpsimd.collective_compute(
            "AllReduce", OP.add, replica_groups=[list(range(NC_))],
            ins=[ar2_in.opt()], outs=[ar2_out.opt()])
        nc.gpsimd.collective_compute(
            "AllToAll", OP.bypass, replica_groups=[list(range(NC_))],
            ins=[a2a_in.opt()], outs=[a2a_out.opt()])
        # receive + transpose-back the 8 peer blocks while the gate chain runs:
        # a2a_out rows [250p, 250p+250) = ori^T for our l-shard, channels of
        # core p; two (125,128) sub-blocks -> PE transpose -> (128,125) lhsT
        rT = []
        for p in range(8):
            for h in range(2):
                rb = work.tile([LC, DL], BF16, tag="rb", bufs=4, name=f"rb{p}_{h}")
                (nc.sync if (2 * p + h) % 2 == 0 else nc.scalar).dma_start(
                    rb[:], a2a_out[250 * p + LC * h:250 * p + LC * (h + 1), :])
                rtp = psum.tile([DL, LC], BF16, tag="mmc", name=f"rtp{p}_{h}")
                nc.tensor.transpose(rtp[:], rb[:], identT[0:LC, 0:LC])
                rt = work.tile([DL, LC], BF16, tag="rt", bufs=16, name=f"rt{p}_{h}")
                if (2 * p + h) % 2 == 0:
                    nc.vector.tensor_copy(rt[:], rtp[:])
                else:
                    nc.scalar.copy(rt[:], rtp[:])
                rT.append(rt)
        stT = cons.tile([128, 32], F32)
        nc.sync.dma_start(stT[:], ar2_out[:])
        muT = cons.tile([128, 16], F32)
        varT = cons.tile([128, 16], F32)
        rstdT = cons.tile([128, 16], BF16)
        rstdTf = cons.tile([128, 16], F32)
        murT = cons.tile([128, 16], F32)
        nc.scalar.mul(muT[:], stT[:, 0:16], 1.0 / DI)
        nc.scalar.mul(varT[:], stT[:, 16:32], 1.0 / DI)
        tmp16 = cons.tile([128, 16], F32)
        nc.vector.tensor_tensor(tmp16[:], muT[:], muT[:], OP.mult)
        nc.vector.tensor_tensor(varT[:], varT[:], tmp16[:], OP.subtract)
        eps = work.tile([128, 1], F32, tag="eps", bufs=1)
        nc.vector.memset(eps[:], 1e-5)
        nc.scalar.activation(rstdTf[:], varT[:], AF.Sqrt, bias=eps[:])
        nc.vector.reciprocal(rstdTf[:], rstdTf[:])
        nc.vector.tensor_copy(rstdT[:], rstdTf[:])
        nc.vector.tensor_tensor(murT[:], muT[:], rstdTf[:], OP.mult)
        murX = work.tile([128, 1], F32, tag="murX", bufs=1)
        nc.vector.reduce_sum(murX[:], murT[:], axis=AX.X)
        mmsm = psum.tile([1, 1], F32, tag="mm", name="mmsm")
        nc.tensor.matmul(mmsm[:], murX[:], ones_colf[:], start=True, stop=True)
        smur = work.tile([1, 1], F32, tag="smur", bufs=1)
        nc.vector.tensor_copy(smur[:], mmsm[:])
        smur_bc = work.tile([128, 1], F32, tag="smurbc", bufs=1)
        nc.gpsimd.partition_broadcast(smur_bc[:], smur[:])
        # S1_d = sum_l ori*r: 16 accumulating matvecs over transposed chunks
        s1dp = psum.tile([DL, 1], F32, tag="mm", name="s1dp")
        for lc in range(16):
            osl = slice(128 * lc, 128 * lc + 128)
            nc.tensor.matmul(s1dp[:], oriT[0:LC, osl], rstdT[0:LC, lc:lc + 1],
                             start=(lc == 0), stop=(lc == 15))
        dump("s1T", s1T[:])
        dump("s2T", s2T[:])
        dump("rstdTf", rstdTf[:])
        dump("smur_bc", smur_bc[:])
        gd = work.tile([DL, 1], F32, tag="gd", bufs=1)
        nc.vector.scalar_tensor_tensor(gd[:], s1dp[:], smur_bc[:], ln_g_s[:],
                                       OP.subtract, OP.mult)
        nc.vector.tensor_tensor(gd[:], gd[:], ln_b_sb[:], OP.add)
        mmg = psum.tile([1, 512], F32, tag="mm", name="mmg")
        nc.tensor.matmul(mmg[:], gd[:], grw_sb[:], start=True, stop=True)
        gpart = work.tile([1, 512], F32, tag="gpart", bufs=1)
        nc.vector.tensor_copy(gpart[:], mmg[:])
        nc.sync.dma_start(ar3_in[:], gpart[:])
        nc.gpsimd.collective_compute(
            "AllReduce", OP.add, replica_groups=[list(range(NC_))],
            ins=[ar3_in.opt()], outs=[ar3_out.opt()])
        ggT0 = work.tile([128, 4], F32, tag="ggT0", bufs=1)
        nc.sync.dma_start(ggT0[:], bass.AP(ar3_out.tensor, 0, [[1, 128], [128, 4]]))
        nc.vector.tensor_tensor(ggT0[:], ggT0[:], grbT_sb[:], OP.add)
        ggT = work.tile([128, 4], BF16, tag="ggT", bufs=1)
        nc.scalar.activation(ggT[:], ggT0[:], AF.Gelu)
        attnF = work.tile([128, 8], F32, tag="attn", bufs=1)
        for j in range(8):
            mma = psum.tile([DL, 1], F32, tag="mm", name=f"mma{j}")
            for k in range(4):
                nc.tensor.matmul(mma[:], cswT_sb[k][:, 128 * j:128 * (j + 1)],
                                 ggT[:, k:k + 1], start=(k == 0), stop=(k == 3))
            nc.scalar.activation(attnF[:, j:j + 1], mma[:], AF.Sigmoid,
                                 bias=csbF_sb[:, j:j + 1])
        dump("attnF", attnF[:])

        # ================= stage 6: local full-d output GEMM ================
        # each core already holds ori^T for its own 250-row l-shard across all
        # 1024 channels (via the early AllToAll); contract locally with the
        # attn-scaled host-built W_comb. No collective after attn.
        wscF = [cons.tile([128, DM], BF16, name=f"wscF{j}") for j in range(8)]
        for j in range(8):
            nc.vector.tensor_scalar_mul(wscF[j][:], wcombF[j][:], attnF[:, j:j + 1])
        for h in range(2):
            mmo = psum.tile([LC, DM], F32, tag="mm", name=f"mmo{h}")
            for p in range(8):
                nc.tensor.matmul(mmo[:], rT[2 * p + h][:], wscF[p][:],
                                 start=(p == 0), stop=(p == 7))
            ob = work.tile([LC, DM], BF16, tag="ob", bufs=2, name=f"ob{h}")
            if h == 0:
                nc.vector.tensor_copy(ob[:], mmo[:])
            else:
                nc.scalar.copy(ob[:], mmo[:])
            (nc.sync if h == 0 else nc.scalar).dma_start(
                out_shard[LC * h:LC * (h + 1), :], ob[:])

    nc.compile()
    return nc


# ---------------------------------------------------------------- host ---

def _prep_inputs(inputs):
    import ml_dtypes
    f = lambda k: np.ascontiguousarray(np.asarray(inputs[k], dtype=np.float32))
    hid = f('hidden_states')[0]
    hidT = np.ascontiguousarray(hid.T)
    in_proj = f('in_proj_w')
    scans = [
        ('conv1d_w', 'conv1d_bias', 'x_proj_w', 'dt_proj_w', 'dt_bias', 'A_log', 'D'),
        ('conv1d_b_w', 'conv1d_b_bias', 'x_proj_b_w', 'dt_proj_b_w', 'dt_b_bias', 'A_b_log', 'D_b'),
        ('conv1d_c_w', 'conv1d_c_bias', 'x_proj_c_w', 'dt_proj_c_w', 'dt_c_bias', 'A_c_log', 'D_c'),
    ]
    ln_g = f('att_ln_g'); ln_b = f('att_ln_b')
    gr_w = f('att_gr_w'); cs_w = f('att_cs_w')
    ow = f('att_out_w'); opw = f('out_proj_w')

    maps = []
    for c in range(NC_):
        dsl = slice(c * DL, (c + 1) * DL)
        m = dict(
            hidT=hidT.astype(ml_dtypes.bfloat16),
            wxT=np.ascontiguousarray(in_proj[c * DL:(c + 1) * DL, :].T).astype(ml_dtypes.bfloat16),
            wzT=np.ascontiguousarray(in_proj[DI + c * DL:DI + (c + 1) * DL, :].T).astype(ml_dtypes.bfloat16),
            ones_colf=np.ones((128, 1), np.float32),
            identT=np.eye(128, dtype=ml_dtypes.bfloat16),
            ln_g_s=np.ascontiguousarray((ln_g[dsl] / L).reshape(DL, 1)),
            ln_b=np.ascontiguousarray(ln_b[dsl].reshape(DL, 1)),
            grw=np.ascontiguousarray(gr_w[:, dsl].T),
            grbT=np.ascontiguousarray(f('att_gr_b').reshape(4, 128).T),
            cswT=np.ascontiguousarray(cs_w.T).astype(ml_dtypes.bfloat16),
            csbF=np.ascontiguousarray(f('att_cs_b').reshape(8, 128).T),
            wcombF=np.ascontiguousarray(ow.T @ opw.T).astype(ml_dtypes.bfloat16),
        )
        for s, keys in enumerate(scans):
            cw, cb, xw, dtw, dtbk, alog, dk = keys
            cwd = np.zeros((4 * DL, DL), np.float32)
            for k in range(4):
                cwd[128 * k:128 * (k + 1), :][np.arange(DL), np.arange(DL)] = f(cw)[dsl, 0, k]
            m[f'convw{s}'] = cwd.astype(ml_dtypes.bfloat16)
            m[f'convb{s}'] = np.ascontiguousarray(f(cb)[dsl].reshape(DL, 1))
            m[f'xwT{s}'] = np.ascontiguousarray(f(xw)[:, dsl].T).astype(ml_dtypes.bfloat16)
            m[f'dtwT{s}'] = np.ascontiguousarray(f(dtw)[dsl, :].T).astype(ml_dtypes.bfloat16)
            m[f'dtb{s}'] = np.ascontiguousarray(f(dtbk)[dsl].reshape(DL, 1))
            m[f'avec{s}'] = np.ascontiguousarray(-np.exp(f(alog)[dsl]))
            dd = np.zeros((DL, DL), np.float32)
            dd[np.arange(DL), np.arange(DL)] = f(dk)[dsl]
            m[f'ddiag{s}'] = dd.astype(ml_dtypes.bfloat16)
        maps.append(m)
    bias_out = f('att_out_b') @ opw.T
    return maps, bias_out


def kernel(**inputs) -> np.ndarray:
    if 'nc' not in _CACHE:
        _CACHE['nc'] = _build()
    nc = _CACHE['nc']
    maps, bias_out = _prep_inputs(inputs)
    trace = bool(os.environ.get('BASS_KERNEL_TRACE'))
    res = run_bass_kernel_spmd(nc, maps, list(range(NC_)), trace=trace)
    _CACHE['last_exec_ns'] = res.exec_time_ns
    _CACHE['last_res'] = res
    shards = [np.asarray(res.results[c]['out_shard']).astype(np.float32) for c in range(NC_)]
    out = np.concatenate(shards, axis=0) + bias_out[None, :]
    return out[None].astype(np.float32)
